# revision 37
# baseline (speedup 1.0000x reference)
"""Trainium2 Bass kernel for BatchNorm2d + 8-head self-attention block.

Reference (per batch element b, all fp32):
    xn = BN_eval(x[b]); t = xn.T
    q/k/v = t @ W.T + b            # [S, 512], 8 heads x 64
    attn  = softmax(q k^T / 8)     # per head
    y[b]  = ((attn v) @ wo.T + bo).T

Sharding: pure data parallel - one batch element per NeuronCore, weights
replicated, no collectives.

Device design (per core), fully in the "transposed" domain:
  - BN folded into QKV weights/biases on host; 1/8 scale folded into wq/bq;
    v bias folded into bo (softmax rows sum to 1).
  - Q^T,K^T [I,S] = wT.T @ x      (x arrives [C,S] - natural rhs)
  - V [S,I]       = x_chunk.T @ wvT, stored interleaved per head with a
    ones column ([128, 8*65]) so the PV matmul (M=65) also produces the
    softmax denominators for free.
  - scores^T per head [t,s]; head pairs row-packed via tile_position
    (0,0)/(64,0), K=64 each - the two matmuls run CONCURRENTLY on separate
    PE row groups (~1 stream of 512 wall-clock for both heads).
  - exp on ScalarE over both heads in one [128,1024] ACTIVATE (no max
    subtraction - scores are in [-3, 3]). ScalarE is the pacing engine:
    64 exps x ~1.11us = 71us is the kernel floor.
  - o^T accumulates over 8 t-chunks (K=128); normalize = approx-reciprocal
    straight off the PSUM denominator row + gpsimd partition-broadcast +
    DVE multiply; y^T = woT.T @ o^T + bo.

Schedule: ONE flat software-pipelined loop over all 64 (slab, head-pair,
t-chunk) chunks. Per chunk i the emission (= Tile scheduler priority) is
exp(i) | PV(i-1) | fillers(i) | scores(i+1), so the PE always has the next
chunk's scores ready before ScalarE needs them, including across head-pair
boundaries (the per-hp-call structure previously stalled ~2.2us at each
boundary). Projections (QKV/out) ride as single-matmul fillers in the PE
slack under each exp.

DMAs are chained in need-order: x[slab0], wq[hp0], wk[hp0], wv, x[slab1],
wq[hp1-3], wk[hp1-3], wo (weights are laid out hp-major on host so per-hp
slices are contiguous). First exp fires ~12us in vs ~22.7us before.

Matmul dtype is fp16: 2-byte weights keep LDWEIGHTS in the PE's background
buffer (hidden behind the previous matmul); fp16's 10-bit mantissa keeps the
end-to-end error ~1e-3 of scale.
"""

import numpy as np

import concourse.bass as bass
import concourse.tile as tile
from concourse import bacc, mybir
from concourse.bass_utils import run_bass_kernel_spmd
from concourse.tile import add_dep_helper

B, C, S = 8, 512, 1024
H, DH, INNER = 8, 64, 512
EPS = 1e-5
SCALE = DH ** (-0.5)
N_CORES = 8
F32 = mybir.dt.float32
F16 = mybir.dt.float16

DT_MM = F16

_CACHE: dict = {}

KC = C // 128      # 4 contraction chunks over channels
IT = INNER // 128  # 4 tiles over inner dim (also head-pair index)
ST = S // 128      # 8 t-chunks
NSLAB = S // 512   # 2 s-slabs
NCHUNK = NSLAB * IT * ST  # 64 pipeline chunks


def build_bass(dt_mm):
    nc = bacc.Bacc("TRN2", target_bir_lowering=False, debug=False,
                   num_devices=N_CORES)

    # x slab-major: [128, NSLAB, KC, 512] so each slab DMA is contiguous
    x_d = nc.dram_tensor("x", [128, NSLAB, KC, 512], dt_mm, kind="ExternalInput")
    # wq/wk laid out hp-major: [128, IT, KC, 128] so per-hp DMA slices are
    # contiguous; wv/wo stay [128, KC, 512].
    wqT_d = nc.dram_tensor("wqT", [128, IT, KC, 128], dt_mm, kind="ExternalInput")
    wkT_d = nc.dram_tensor("wkT", [128, IT, KC, 128], dt_mm, kind="ExternalInput")
    wvT_d = nc.dram_tensor("wvT", [128, KC, 512], dt_mm, kind="ExternalInput")
    woT_d = nc.dram_tensor("woT", [128, KC, 512], dt_mm, kind="ExternalInput")
    # bq | bk | bo packed on host as [128, 12] (col t+0/4/8 = vec[t*128+p])
    bias_d = nc.dram_tensor("bias_pack", [128, 3 * IT], F32, kind="ExternalInput")
    y_d = nc.dram_tensor("y", [C, S], F32, kind="ExternalOutput")

    with tile.TileContext(nc) as tc:
        with (
            tc.tile_pool(name="persist", bufs=1) as persist,
            tc.tile_pool(name="stage", bufs=2) as stage,
            tc.tile_pool(name="out", bufs=4) as outp,
            # 48 et buffers: exps 0-47 have no write-after-write on their et
            # slot, so the Tile lowering can attach the scores-done wait to
            # the ACTIVATE itself instead of spending a standalone ~130ns
            # EVENT_SEMAPHORE on the Scalar queue per exp.
            tc.tile_pool(name="et", bufs=48) as etp,
            tc.tile_pool(name="norm", bufs=2) as normp,
            # psA: tag "acc" = 2 rotating projection-accumulator banks;
            # tag "po" = 2 banks for the attention accumulators (own ring so
            # projection churn never lands on a live po, and vice versa)
            tc.tile_pool(name="psA", bufs=2, space="PSUM") as psA,
            tc.tile_pool(name="psS", bufs=2, space="PSUM") as psS,
        ):
            xr = persist.tile([128, NSLAB, KC, 512], dt_mm, tag="xr", name="xr")
            wqr = persist.tile([128, IT, KC, 128], dt_mm, tag="wqr", name="wqr")
            wkr = persist.tile([128, IT, KC, 128], dt_mm, tag="wkr", name="wkr")
            wvr = persist.tile([128, KC, 512], dt_mm, tag="wvr", name="wvr")
            wor = persist.tile([128, KC, 512], dt_mm, tag="wor", name="wor")

            # ---- chained input DMAs in need-order ----
            dmas = []
            dmas.append(nc.sync.dma_start(wqr[:, 0], wqT_d[:, 0]))
            dmas.append(nc.sync.dma_start(xr[:, 0], x_d[:, 0]))
            dmas.append(nc.sync.dma_start(wkr[:, 0], wkT_d[:, 0]))
            dmas.append(nc.sync.dma_start(wvr[:], wvT_d[:]))
            dmas.append(nc.sync.dma_start(xr[:, 1], x_d[:, 1]))
            dmas.append(nc.sync.dma_start(wqr[:, 1:4], wqT_d[:, 1:4]))
            dmas.append(nc.sync.dma_start(wkr[:, 1:4], wkT_d[:, 1:4]))
            dmas.append(nc.sync.dma_start(wor[:], woT_d[:]))
            for a, b in zip(dmas[1:], dmas):
                add_dep_helper(a.ins, b.ins, sync=False, reason="dma priority")

            bias_sb = persist.tile([128, 3 * IT], F32, tag="bias")
            nc.gpsimd.dma_start(bias_sb[:], bias_d[:])
            bq_sb = bias_sb[:, 0:IT]
            bk_sb = bias_sb[:, IT:2 * IT]
            bo_sb = bias_sb[:, 2 * IT:3 * IT]

            ones_sb = persist.tile([128, H], F32, tag="ones")
            nc.vector.memset(ones_sb[:], 1.0)
            ones_row = persist.tile([1, 64], F32, tag="ones_row")
            nc.vector.memset(ones_row[:], 1.0)

            # ---- PE warmup: keep the PE busy through the initial DMA wait;
            # needs ~3.4us of sustained activity to trip the HAM clock-gate
            # to 8/8 before real work ----
            warm_sb = stage.tile([128, 256], dt_mm, tag="warm", bufs=1)
            nc.vector.memset(warm_sb[:], 0.0)
            warm_ps = psA.tile([128, 256], F32, tag="acc", name="warm_ps")
            for wi in range(12):
                nc.tensor.matmul(warm_ps[:], warm_sb[:, 0:128], warm_sb[:],
                                 start=(wi == 0), stop=(wi == 11))

            # ---- persistent per-slab outputs ----
            qT = [[persist.tile([128, 512], dt_mm, tag=f"qT{i}{s}",
                                name=f"qT{i}{s}") for s in range(NSLAB)]
                  for i in range(IT)]
            kT = [[persist.tile([128, 512], dt_mm, tag=f"kT{i}{s}",
                                name=f"kT{i}{s}") for s in range(NSLAB)]
                  for i in range(IT)]
            oT = [[persist.tile([128, 512], dt_mm, tag=f"oT{i}{s}",
                                name=f"oT{i}{s}") for s in range(NSLAB)]
                  for i in range(IT)]
            v_sb = [persist.tile([128, H * 65], dt_mm, tag=f"v{t}",
                                 name=f"v{t}") for t in range(ST)]

            def group_thunks(n_mm, emit_mm, evac):
                """n_mm single-matmul thunks accumulating into one psA bank;
                the first allocates the bank, the last appends the evacuation."""
                box = []

                def mk(i):
                    def t():
                        if i == 0:
                            box.append(psA.tile([128, 512], F32,
                                                tag="acc", name="acc"))
                        emit_mm(box[0], i)
                        if i == n_mm - 1:
                            evac(box[0])
                    return t

                return [mk(i) for i in range(n_mm)]

            def qk_thunks(w, bias, dst, hp, sl):
                def emit_mm(ps, kc):
                    nc.tensor.matmul(
                        ps[:],
                        w[:, hp, kc, :],
                        xr[:, sl, kc, :],
                        start=(kc == 0), stop=(kc == KC - 1),
                    )

                def evac(ps):
                    if bias is None:
                        # k needs no bias: a bias-on-k term is constant along
                        # the softmax axis (t) and cancels; plain copy is ~2x
                        # faster on DVE than tensor_scalar.
                        if hp == 0 and sl == 0:
                            # lead-in fast path: scores(0) only needs the
                            # first 128 t-columns; evac those first so the
                            # first exp is not gated on the full copy
                            nc.vector.tensor_copy(dst[hp][sl][:, 0:128],
                                                  ps[:, 0:128])
                            nc.vector.tensor_copy(dst[hp][sl][:, 128:512],
                                                  ps[:, 128:512])
                        else:
                            nc.vector.tensor_copy(dst[hp][sl][:], ps[:])
                    else:
                        nc.vector.tensor_scalar_add(
                            dst[hp][sl][:], ps[:], bias[:, hp:hp + 1]
                        )

                return group_thunks(KC, emit_mm, evac)

            def v_thunks(tc_):
                def emit_mm(ps, kc):
                    nc.tensor.matmul(
                        ps[:],
                        xr[:, tc_ // 4, kc, (tc_ % 4) * 128:(tc_ % 4 + 1) * 128],
                        wvr[:, kc, :],
                        start=(kc == 0), stop=(kc == KC - 1),
                    )

                def evac(ps):
                    vv = v_sb[tc_][:].rearrange("p (h m) -> p h m", h=H)
                    nc.vector.tensor_copy(
                        vv[:, :, 0:64], ps[:].rearrange("p (h m) -> p h m", h=H)
                    )
                    nc.vector.tensor_copy(vv[:, :, 64:65], ones_sb[:, :, None])

                return group_thunks(KC, emit_mm, evac)

            y_part = [persist.tile([128, 512], F32, tag=f"yp{ct}",
                                   name=f"yp{ct}") for ct in range(IT)]

            def op_thunks(sl, ct, order=None):
                order = list(order or range(IT))

                def emit_mm(ps, j):
                    ic = order[j]
                    nc.tensor.matmul(
                        ps[:],
                        wor[:, ic, ct * 128:(ct + 1) * 128],
                        oT[ic][sl][:],
                        start=(j == 0), stop=(j == IT - 1),
                    )

                def evac(ps):
                    ysb = outp.tile([128, 512], F32, tag="ysb", name="ysb")
                    nc.vector.tensor_scalar_add(ysb[:], ps[:],
                                                bo_sb[:, ct:ct + 1])
                    nc.sync.dma_start(
                        y_d[ct * 128:(ct + 1) * 128,
                            sl * 512:(sl + 1) * 512],
                        ysb[:],
                    )

                return group_thunks(IT, emit_mm, evac)

            def op_partial_thunks(ct):
                # ic 0..2 of the sl=1 projection, banked into SBUF (+bias)
                def emit_mm(ps, ic):
                    nc.tensor.matmul(
                        ps[:],
                        wor[:, ic, ct * 128:(ct + 1) * 128],
                        oT[ic][1][:],
                        start=(ic == 0), stop=(ic == IT - 2),
                    )

                def evac(ps):
                    nc.vector.tensor_scalar_add(y_part[ct][:], ps[:],
                                                bo_sb[:, ct:ct + 1])

                return group_thunks(IT - 1, emit_mm, evac)

            def op_final(ct):
                ps = psA.tile([128, 512], F32, tag="acc", name="acc")
                nc.tensor.matmul(
                    ps[:],
                    wor[:, IT - 1, ct * 128:(ct + 1) * 128],
                    oT[IT - 1][1][:],
                    start=True, stop=True,
                )
                ysb = outp.tile([128, 512], F32, tag="ysb", name="ysb")
                nc.vector.tensor_add(ysb[:], y_part[ct][:], ps[:])
                nc.sync.dma_start(
                    y_d[ct * 128:(ct + 1) * 128, 512:1024], ysb[:],
                )

            # ---- flat pipelined chunk machinery ----
            def decode(i):
                return i // 32, (i % 32) // 8, i % 8  # sl, hp, tc

            pss_of = {}
            et_of = {}
            po_of = {}  # phase index (i//8) -> (po0, po1)

            def emit_scores(i):
                sl, hp, tc_ = decode(i)
                pss = psS.tile([128, 1024], F32, tag="psS", name="psS")
                ksl, kcol = tc_ // 4, (tc_ % 4) * 128
                nc.tensor.matmul(
                    pss[:, 0:512],
                    kT[hp][ksl][0:64, kcol:kcol + 128],
                    qT[hp][sl][0:64, :],
                    start=True, stop=True, tile_position=(0, 0),
                )
                nc.tensor.matmul(
                    pss[:, 512:1024],
                    kT[hp][ksl][64:128, kcol:kcol + 128],
                    qT[hp][sl][64:128, :],
                    start=True, stop=True, tile_position=(64, 0),
                )
                pss_of[i] = pss

            def emit_exp(i):
                et = etp.tile([128, 1024], dt_mm, tag="et", name="et")
                nc.scalar.activation(
                    et[:], pss_of.pop(i)[:], mybir.ActivationFunctionType.Exp
                )
                et_of[i] = et

            def emit_pv(i):
                sl, hp, tc_ = decode(i)
                ph = i // 8
                if tc_ == 0:
                    po_of[ph] = (
                        psA.tile([65, 512], F32, tag="po", bufs=2, name="po0"),
                        psA.tile([65, 512], F32, tag="po", bufs=2, name="po1"),
                    )
                po0, po1 = po_of[ph]
                et = et_of.pop(i)
                h0, h1 = 2 * hp, 2 * hp + 1
                nc.tensor.matmul(
                    po0[:], v_sb[tc_][:, h0 * 65:(h0 + 1) * 65],
                    et[:, 0:512],
                    start=(tc_ == 0), stop=(tc_ == ST - 1),
                )
                nc.tensor.matmul(
                    po1[:], v_sb[tc_][:, h1 * 65:(h1 + 1) * 65],
                    et[:, 512:1024],
                    start=(tc_ == 0), stop=(tc_ == ST - 1),
                )
                if tc_ == ST - 1:
                    emit_normalize(sl, hp, po_of.pop(ph), last=(ph == 7))

            def emit_normalize(sl, hp, pos, last=False):
                # Evacuate po (both the 64 value rows and the denominator
                # row) to SBUF FIRST: this releases the po PSUM banks ~1.5us
                # after the phase ends, so the 2-slot po ring never stalls
                # the next phase's PV. The recip/broadcast/multiply then run
                # entirely from SBUF, off the po critical path. The last
                # phase keeps the direct-from-PSUM multiply (shorter tail).
                if last:
                    # tail latency path: both halves' denominators through
                    # ONE recip + ONE gpsimd broadcast (saves ~1.2us serial)
                    drow2 = normp.tile([1, 1024], F32, tag="drow2", bufs=1,
                                       name="drow2")
                    nc.vector.tensor_copy(drow2[:, 0:512], pos[0][64:65, :])
                    nc.vector.tensor_copy(drow2[:, 512:1024], pos[1][64:65, :])
                    rrow2 = normp.tile([1, 1024], F32, tag="rrow2", bufs=1,
                                       name="rrow2")
                    nc.vector.reciprocal_approx_fast(rrow2[:], drow2[:])
                    rbc2 = normp.tile([64, 1024], F32, tag="rbc2", bufs=1,
                                      name="rbc2")
                    nc.gpsimd.partition_broadcast(rbc2[:], rrow2[:])
                    for half in range(2):
                        nc.vector.tensor_mul(
                            oT[hp][sl][half * 64:(half + 1) * 64, :],
                            pos[half][0:64, :],
                            rbc2[:, half * 512:(half + 1) * 512],
                        )
                    return
                drows, oraws = [], []
                for half, po in ((0, pos[0]), (1, pos[1])):
                    oraw = normp.tile([64, 512], F32, tag="oraw",
                                      bufs=4, name="oraw")
                    nc.vector.tensor_copy(oraw[:], po[0:64, :])
                    oraws.append(oraw)
                    drow = normp.tile([1, 512], F32, tag="drow", name="drow")
                    nc.vector.tensor_copy(drow[:], po[64:65, :])
                    drows.append(drow)
                rbcs = []
                for half in range(2):
                    rrow = normp.tile([1, 512], F32, tag="rrow", name="rrow")
                    nc.vector.reciprocal_approx_fast(rrow[:], drows[half][:])
                    rbc = normp.tile([64, 512], F32, tag="rbc", name="rbc")
                    nc.gpsimd.partition_broadcast(rbc[:], rrow[:])
                    rbcs.append(rbc)
                for half in range(2):
                    nc.vector.tensor_mul(
                        oT[hp][sl][half * 64:(half + 1) * 64, :],
                        oraws[half][:],
                        rbcs[half][:],
                    )

            # ---- filler assignment: chunk index -> list of 1-MM thunks ----
            fill = [[] for _ in range(NCHUNK)]

            def assign(thunks, chunks):
                """Spread thunks over the given chunk indices round-robin-ish,
                preserving thunk order."""
                nper = -(-len(thunks) // len(chunks))
                it_ = iter(thunks)
                done = False
                for c in chunks:
                    for _ in range(nper):
                        try:
                            fill[c].append(next(it_))
                        except StopIteration:
                            done = True
                            break
                    if done:
                        break

            q_th = {(hp, sl): qk_thunks(wqr, bq_sb, qT, hp, sl)
                    for hp in range(IT) for sl in range(NSLAB)}
            k_th = {(hp, ksl): qk_thunks(wkr, None, kT, hp, ksl)
                    for hp in range(IT) for ksl in range(NSLAB)}

            def dummy_thunk():
                # 1-matmul psA allocation (~60ns): ring churn so the next
                # phase's po accumulators land on promptly-freed slots
                # instead of the previous phase's not-yet-normalized po.
                def t():
                    ps = psA.tile([128, 64], F32, tag="acc", name="dummy")
                    nc.tensor.matmul(ps[:], warm_sb[:, 0:128],
                                     warm_sb[:, 0:64], start=True, stop=True)
                return t

            # v projections: v0 lands within chunk 0's exp window (PV(0)
            # reads it at exp(0)-end); v_sb[t] one chunk ahead of its PV
            assign(v_thunks(0), [0])
            for t in range(1, 8):
                assign(v_thunks(t), [t - 1])
            assign(k_th[(0, 1)], [0, 1])       # needed by scores(4)
            assign(q_th[(1, 0)], [2, 3])       # needed by scores(8)
            assign(k_th[(1, 0)], [4, 5])       # needed by scores(8)
            assign(k_th[(1, 1)], [6, 7])       # needed by scores(12)
            assign(q_th[(2, 0)], [8, 9, 10, 11])
            assign(k_th[(2, 0)], [8, 9, 10, 11])
            assign(k_th[(2, 1)], [12, 13])
            assign(q_th[(3, 0)], [14, 15, 16, 17])
            assign(k_th[(3, 0)], [14, 15, 16, 17])
            assign(k_th[(3, 1)], [18, 19])
            assign(q_th[(0, 1)], [20, 21, 22, 23])
            assign(q_th[(1, 1)], [24, 25, 26, 27])
            assign(q_th[(2, 1)], [28, 29, 30, 31])
            # out-projection for slab 0: oT[*][0] complete ~chunk 33.5
            c0 = op_thunks(0, 0)
            c1 = op_thunks(0, 1)
            c2 = op_thunks(0, 2)
            c3 = op_thunks(0, 3)
            op0 = (c0[:3] + c1[:3] + [c0[3], c1[3]]
                   + c2[:3] + c3[:3] + [c2[3], c3[3]])
            assign(op0, [35, 36, 37, 38, 39])
            assign(q_th[(3, 1)], [40, 41, 42, 43])
            op1p = (op_partial_thunks(0) + op_partial_thunks(1)
                    + op_partial_thunks(2) + op_partial_thunks(3))
            assign(op1p, [58, 59, 60, 61, 62, 63])

            # ---- lead-in: q00, k00, scores(0) ----
            for t in q_th[(0, 0)]:
                t()
            for t in k_th[(0, 0)]:
                t()
            emit_scores(0)

            # ---- the flat pipelined loop: scores(i+1) right after exp(i)
            # so both halves of the pair sit at the head of the PE order ----
            for i in range(NCHUNK):
                emit_exp(i)
                if i + 1 < NCHUNK:
                    emit_scores(i + 1)
                if i >= 1:
                    emit_pv(i - 1)
                for t in fill[i]:
                    t()
            last_et = et_of[NCHUNK - 1]
            emit_pv(NCHUNK - 1)
            # keep the PE busy through the final normalize chain so HAM
            # stays at 8/8 for the op_final matmuls; gate on the last et so
            # the scheduler cannot hoist these earlier
            tail_warm = psS.tile([128, 256], F32, tag="psS", name="tail_warm")
            for wi in range(22):
                nc.tensor.matmul(tail_warm[:, 0:256], warm_sb[:, 0:128],
                                 last_et[:, (wi % 10) * 64:(wi % 10) * 64 + 256],
                                 start=(wi == 0), stop=(wi == 21))
            for ct in range(IT):
                op_final(ct)

    nc.compile()
    return nc


def prep_host(inputs, dt_mm):
    """Fold BN + scale + v-bias into effective weights (fp32 numpy)."""
    x = np.asarray(inputs["x"], dtype=np.float32)
    g = np.asarray(inputs["bn_gamma"], dtype=np.float32)
    be = np.asarray(inputs["bn_beta"], dtype=np.float32)
    mu = np.asarray(inputs["bn_mean"], dtype=np.float32)
    var = np.asarray(inputs["bn_var"], dtype=np.float32)
    wq = np.asarray(inputs["wq"], dtype=np.float32)
    bq = np.asarray(inputs["bq"], dtype=np.float32)
    wk = np.asarray(inputs["wk"], dtype=np.float32)
    bk = np.asarray(inputs["bk"], dtype=np.float32)
    wv = np.asarray(inputs["wv"], dtype=np.float32)
    bv = np.asarray(inputs["bv"], dtype=np.float32)
    wo = np.asarray(inputs["wo"], dtype=np.float32)
    bo = np.asarray(inputs["bo"], dtype=np.float32)

    a = g / np.sqrt(var + EPS)          # [C]
    bvec = be - mu * a                  # [C]

    wq_eff = wq * a[None, :] * SCALE
    bq_eff = (bq + wq @ bvec) * SCALE
    wk_eff = wk * a[None, :]
    bk_eff = bk + wk @ bvec
    wv_eff = wv * a[None, :]
    bv_eff = bv + wv @ bvec
    wo_eff = wo
    bo_eff = bo + wo @ bv_eff           # v bias rides through softmax

    bias_pack = np.concatenate(
        [bq_eff.reshape(IT, 128).T, bk_eff.reshape(IT, 128).T,
         bo_eff.reshape(IT, 128).T], axis=1
    ).astype(np.float32)

    np_dt = np.float16

    def dev_layout(a_):
        # [C_or_I, N] -> [128, KC, N]: partition p holds rows {k*128+p}
        return np.ascontiguousarray(
            a_.reshape(KC, 128, a_.shape[1]).transpose(1, 0, 2).astype(np_dt))

    def dev_layout_hp(a_):
        # w.T [C, I] -> [128, IT, KC, 128]: [p, hp, kc, m] = a_[kc*128+p, hp*128+m]
        return np.ascontiguousarray(
            a_.reshape(KC, 128, IT, 128).transpose(1, 2, 0, 3).astype(np_dt))

    def dev_layout_x(a_):
        # x [C, S] -> [128, NSLAB, KC, 512]: [p, sl, kc, s] = a_[kc*128+p, sl*512+s]
        return np.ascontiguousarray(
            a_.reshape(KC, 128, NSLAB, 512).transpose(1, 2, 0, 3).astype(np_dt))

    wq_l = dev_layout_hp(wq_eff.T)
    wk_l = dev_layout_hp(wk_eff.T)
    wv_l = dev_layout(wv_eff.T)
    wo_l = dev_layout(wo_eff.T)
    per_core = []
    for b in range(B):
        per_core.append({
            "x": dev_layout_x(x[b, :, :, 0]),
            "wqT": wq_l,
            "wkT": wk_l,
            "wvT": wv_l,
            "woT": wo_l,
            "bias_pack": np.ascontiguousarray(bias_pack),
        })
    return per_core


def _get_nc(dt_mm):
    key = str(dt_mm)
    if key not in _CACHE:
        _CACHE[key] = build_bass(dt_mm)
    return _CACHE[key]


def kernel(**inputs):
    nc = _get_nc(DT_MM)
    in_maps = prep_host(inputs, DT_MM)
    res = run_bass_kernel_spmd(nc, in_maps, list(range(N_CORES)))
    y = np.stack([res.results[c]["y"] for c in range(N_CORES)], axis=0)
    return y[..., None].astype(np.float32)


def run_traced(**inputs):
    """Like kernel() but with NTFF profiling; returns (y, results, tmpdir)."""
    nc = _get_nc(DT_MM)
    in_maps = prep_host(inputs, DT_MM)
    import tempfile
    tmpdir = tempfile.mkdtemp(prefix="mha_trace_")
    res = run_bass_kernel_spmd(
        nc, in_maps, list(range(N_CORES)), trace=True, tmpdir=tmpdir
    )
    y = np.stack([res.results[c]["y"] for c in range(N_CORES)], axis=0)
    return y[..., None].astype(np.float32), res, tmpdir
